# revision 19
# baseline (speedup 1.0000x reference)
"""Trainium2 Bass kernel for channel attention (XCA-style) module.

Computation (per batch b):
  qkv = w_qkv @ x          (1x1 conv, 192 -> 576 ch)
  qkv = dwconv3x3(qkv)     (depthwise, pad 1)
  q,k,v = split; per head (48 ch): l2-normalize q,k along spatial,
  attn = softmax(temp * q_hat k_hat^T); out = attn @ v
  out = w_proj @ out       (1x1 conv)

Sharding: 8 cores = 4 batches x 2 head-pairs. Each core handles one batch and
2 of the 4 heads (288 of 576 qkv channels), producing a partial projection
output [192, 16384]; host sums the two partials per batch.

q/k path runs in fp8 (e4m3) with DoubleRowSwInterleave matmuls: the 1x1 GEMM
folds its two 96-channel K-halves into one DR matmul per chunk, and the 3x3
depthwise conv runs as 5 DR tap-pair matmuls per chunk against a gapless
(row stride 128) fp8 pad buffer. The gapless layout makes dx=+-1 taps wrap
2 edge columns into the neighboring row; the resulting q/k noise is tiny and
washes out in the 16384-wide gram contraction. v path stays bf16 for accuracy
(its error passes straight to the output).

Layouts (per core):
  q/k groups: 96 channels (h0 0:48, h1 64:112) on a 128-padded partition dim
  v group: same 128-partition padding
  qkT store: per 128-pixel subtile s, columns [s*256, s*256+256) =
             [q_h0(48) pad k_h0(48) pad q_h1(48) pad k_h1(48) pad]
  gram per head: [112, 112] from 64-padded stacked [q_h | k_h] blocks.
"""

import sys

sys.path.insert(0, "/opt/trn_rl_repo")

import numpy as np
import ml_dtypes

import concourse.bass as bass
import concourse.mybir as mybir
from concourse import tile
from concourse.bass_types import AP
from concourse.bass_utils import run_bass_kernel_spmd

F32 = mybir.dt.float32
BF16 = mybir.dt.bfloat16
F8 = mybir.dt.float8e4
SWI = mybir.MatmulPerfMode.DoubleRowSwInterleave

DIM = 192
HEADS = 4
B = 4
HH = 128
WW = 128
NPIX = HH * WW          # 16384
GCH = 96                # q/k channels per core (2 heads x 48)
NCHUNK = 32             # 512-pixel chunks
CH = 512                # chunk size (4 image rows)
EPS = 1e-12
RS = 132                # v pad row stride (2 pad cols each side)
G8 = 128                # fp8 pad guard prefix/suffix (elements)
# fp8 pad: flat(pr, col) = G8 + pr*128 + col, pr 0..129 (pr0/pr129 zero rows)
PAD8_LEN = G8 + 130 * 128 + G8

# DR tap pairs for the fp8 dwconv: (tA, tB, offA, delta); tB None = dummy
# off(t) = dy*128 + dx for t = 3*(dy+1) + (dx+1)
DR_PAIRS = [
    (0, 2, -129, 2),
    (3, 5, -1, 2),
    (6, 8, 127, 2),
    (1, 7, -128, 256),
    (4, None, 0, 2),
]
# v-group tap split (RS=132 padded bf16 buffer)
PE_TAPS_V = [0, 2, 3, 5, 6, 7, 8]
DVE_TAPS_V = [1, 4]
PE_TAPS_V_EVEN = [0, 2, 3, 5, 6, 8]
DVE_TAPS_V_EVEN = [1, 4, 7]


def _split_multiwait(nc):
    """walrus in this env only encodes one sem-wait per instruction; hoist
    extra waits into single-wait NoOps placed just before the instruction."""
    for f in nc.m.functions:
        for bb in f.blocks:
            insts = bb.instructions
            i = 0
            while i < len(insts):
                inst = insts[i]
                si = getattr(inst, "sync_info", None)
                ow = list(si.on_wait) if (si is not None and si.on_wait) else []
                if len(ow) > 1:
                    nops = []
                    for w in ow[:-1]:
                        nops.append(
                            mybir.InstNoOp(
                                name=nc.get_next_instruction_name(),
                                sync_info=mybir.SyncInfo(on_wait=[w], on_update=[]),
                                bass_nofuse=True,
                                engine=inst.engine,
                            )
                        )
                    inst.sync_info = mybir.SyncInfo(
                        on_wait=[ow[-1]], on_update=list(si.on_update)
                    )
                    insts[i:i] = nops
                    i += len(nops)
                i += 1


def _build_kernel():
    nc = bass.Bass("TRN2", target_bir_lowering=False, debug=False, num_devices=8)

    # ---- DRAM I/O ----
    # qk path: fp8 x, both halves side by side [96, 2*16384]
    x8 = nc.dram_tensor("x8", [GCH, 2 * NPIX], F8, kind="ExternalInput")
    # v path: bf16 x halves
    x_half = [
        nc.dram_tensor(f"x{h}", [GCH, NPIX], BF16, kind="ExternalInput")
        for h in range(2)
    ]
    # qk GEMM weights, SwInterleaved: [96, 2 groups * 256]
    wq8 = nc.dram_tensor("wq8", [GCH, 512], F8, kind="ExternalInput")
    # v GEMM weights bf16: wqT v-columns only
    wqv0 = nc.dram_tensor("wqv0", [GCH, 128], BF16, kind="ExternalInput")
    wqv1 = nc.dram_tensor("wqv1", [GCH, 128], BF16, kind="ExternalInput")
    # qk dwconv DR pair weights: [128, 2 groups * 5 pairs * 256]
    wd8 = nc.dram_tensor("wd8", [GCH, 2 * 5 * 256], F8, kind="ExternalInput")
    # v dwconv diag weights bf16: [128, 9*128]
    wdiag = nc.dram_tensor("wdiag", [128, 9 * 128], BF16, kind="ExternalInput")
    wpT = nc.dram_tensor("wpT", [GCH, DIM], BF16, kind="ExternalInput")
    wtap = nc.dram_tensor("wtap", [128, 9], F32, kind="ExternalInput")
    tempv = nc.dram_tensor("tempv", [48, 2], F32, kind="ExternalInput")
    id128 = nc.dram_tensor("id128", [128, 128], F32, kind="ExternalInput")
    outp = nc.dram_tensor("outp", [DIM, NPIX], BF16, kind="ExternalOutput")

    with tile.TileContext(nc) as tc:
        with (
            tc.tile_pool(name="persist", bufs=1) as pp,
            tc.tile_pool(name="scratch", bufs=6) as sp,
            tc.tile_pool(name="stage", bufs=4) as stp,
            tc.tile_pool(name="xstream", bufs=8) as xsp,
            tc.tile_pool(name="ps_raw", bufs=3, space="PSUM") as ps_raw,
            tc.tile_pool(name="ps_dw", bufs=2, space="PSUM") as ps_dw,
            tc.tile_pool(name="ps_tr", bufs=2, space="PSUM") as ps_tr,
            tc.tile_pool(name="ps_gram", bufs=1, space="PSUM") as ps_gram,
        ):
            # ---- persistent SBUF ----
            # first-needed weights on the sync queue; rest via Pool soft-DGE
            wq8t = pp.tile([GCH, 512], F8, tag="wq8t")
            nc.sync.dma_start(wq8t[:], wq8[:])
            wd8t = pp.tile([GCH, 2 * 5 * 256], F8, tag="wd8t")
            nc.gpsimd.dma_start(wd8t[:], wd8[:])
            wv0 = pp.tile([GCH, 128], BF16, tag="wv0")
            nc.gpsimd.dma_start(wv0[:], wqv0[:])
            wv1 = pp.tile([GCH, 128], BF16, tag="wv1")
            nc.gpsimd.dma_start(wv1[:], wqv1[:])
            wd = pp.tile([128, 9 * 128], BF16, tag="wd")
            nc.gpsimd.dma_start(wd[:], wdiag[:])
            wph = []
            for h in range(2):
                t = pp.tile([48, DIM], BF16, tag=f"wp{h}", name=f"wpt{h}")
                nc.gpsimd.dma_start(t[:], wpT[48 * h : 48 * h + 48, :])
                wph.append(t)
            wt = pp.tile([128, 9], F32, tag="wt")
            nc.gpsimd.dma_start(wt[:], wtap[:])
            tv = pp.tile([48, 2], F32, tag="tv")
            nc.gpsimd.dma_start(tv[:], tempv[:])
            idf = pp.tile([128, 128], F32, tag="idf")
            nc.gpsimd.dma_start(idf[:], id128[:])
            idb = pp.tile([128, 128], BF16, tag="idb")
            nc.vector.tensor_copy(idb[:], idf[:])

            # preload the sqrt/exp activation table sets during the startup
            # window so no ACT_TABLE_LOAD fires in the serial k->v finalize
            dum = pp.tile([1, 4], F32, tag="dum")
            nc.vector.memset(dum[:], 1.0)
            nc.scalar.sqrt(dum[:, 2:3], dum[:, 0:1])
            nc.scalar.activation(
                dum[:, 3:4], dum[:, 0:1], mybir.ActivationFunctionType.Exp
            )

            # fp8 pad buffers for q and k groups: gapless rows (stride 128),
            # pr0/pr129 zero rows, 128-elem zero guards both ends
            pad8s = []
            for pi in range(2):
                p8 = pp.tile([GCH, PAD8_LEN], F8, tag=f"pad8_{pi}",
                             name=f"pad8_{pi}")
                nc.vector.memset(p8[:, 0 : G8 + 128], 0.0)
                nc.vector.memset(p8[:, G8 + 129 * 128 :], 0.0)
                pad8s.append(p8)

            # bf16 padded image buffer for v: 130 rows x 132 cols + tail
            padf = pp.tile([128, 130 * RS + 4], BF16, tag="padv")
            pad = padf[:, 0 : 130 * RS].rearrange("p (r x) -> p r x", r=130, x=RS)
            nc.vector.memset(pad[:, 0, :], 0.0)
            nc.vector.memset(pad[:, 129, :], 0.0)
            nc.vector.memset(pad[:, :, 0:2], 0.0)
            nc.vector.memset(pad[:, :, 130:132], 0.0)
            nc.vector.memset(padf[:, 130 * RS :], 0.0)

            # transposed q/k store: per 128-pixel subtile s, 256 columns:
            # [q_h0(48) pad(16) k_h0(48) pad(16) q_h1(48) pad(16) k_h1(48) pad(16)]
            qkT = pp.tile([128, 128 * 256], BF16, tag="qkT")
            qkTr = qkT[:].rearrange("p (s b c) -> p s b c", s=128, b=2, c=128)
            qkTp = qkT[:].rearrange("p (s b c) -> p s b c", s=128, b=4, c=64)
            nc.gpsimd.memset(qkTp[:, :, :, 48:64], 0.0)

            # gram accumulators: both heads share one PSUM bank; FD=112
            gramt = ps_gram.tile([112, 192], F32, tag="gram")
            grams = [gramt[:, 96 * h : 96 * (h + 1)] for h in range(2)]
            # diag-extraction mask for the [q48|k48]-packed gram columns
            idm = pp.tile([112, 96], F32, tag="idm")
            nc.vector.tensor_copy(idm[:, 0:48], idf[0:112, 0:48])
            nc.vector.tensor_copy(idm[:, 48:96], idf[0:112, 64:112])

            # M^T for final GEMM: [128(d-padded), 192(o)] bf16
            mt = pp.tile([128, DIM], BF16, tag="mt")
            nc.gpsimd.memset(mt[:], 0.0)

            x8r = x8[:].rearrange("p (h n) -> p h n", h=2)

            # ================= fused sweep emission =================
            # A(g): 1x1 GEMM chunk into pad buffer; B(g): dwconv+downstream.
            # Emission order A(q); B(q)+A(k); B(k)+A(v); finalize; B(v) keeps
            # the PE busy with taps while ACT fills the next group's pad.
            def emitA8(g, i, dq=None):
                p8 = pad8s[g]
                xt8 = xsp.tile([GCH, 1024], F8, tag="xs8", name=f"x8_{g}_{i}")
                (dq or nc.sync).dma_start(
                    xt8[:].rearrange("p (h n) -> p h n", h=2),
                    x8r[:, :, CH * i : CH * (i + 1)],
                )
                praw = ps_raw.tile([128, CH], F32, tag="praw",
                                   name=f"praw8_{g}_{i}")
                nc.tensor.matmul(
                    praw[:],
                    wq8t[:, 256 * g : 256 * g + 256],
                    xt8[:].rearrange("p (h n) -> p h n", h=2),
                    start=True, stop=True, perf_mode=SWI,
                )
                base = G8 + (4 * i + 1) * 128
                nc.scalar.copy(p8[:, base : base + 512], praw[0:GCH, :])

            def emitB8(g, ip):
                p8v = pad8s[g][:]
                pdws = [
                    ps_dw.tile([128, CH], F32, tag="pdw", name=f"pdw_{g}_{i}")
                    for i in (2 * ip, 2 * ip + 1)
                ]
                # pair-outer so consecutive matmuls share stationary
                for j, (ta, tb, offa, dlt) in enumerate(DR_PAIRS):
                    for k, i in enumerate((2 * ip, 2 * ip + 1)):
                        base = G8 + (4 * i + 1) * 128
                        rhs = AP(
                            p8v.tensor,
                            p8v.offset + base + offa,
                            [tuple(p8v.ap[0]), (dlt, 2), (1, 512)],
                        )
                        nc.tensor.matmul(
                            pdws[k][:],
                            wd8t[:, (5 * g + j) * 256 : (5 * g + j + 1) * 256],
                            rhs,
                            start=(j == 0), stop=(j == 4),
                            perf_mode=SWI,
                        )

                for k, i in enumerate((2 * ip, 2 * ip + 1)):
                    # evac bf16, transpose 4x blocks, store into qkT slots
                    dsc = sp.tile([GCH, CH], BF16, tag="dsc",
                                  name=f"dsc_{g}_{i}")
                    nc.vector.tensor_copy(dsc[:], pdws[k][0:GCH, :])
                    ptr = ps_tr.tile([128, 4 * GCH], BF16, tag="ptr",
                                     name=f"ptr_{g}_{i}")
                    for j in range(4):
                        nc.tensor.transpose(
                            ptr[:, 96 * j : 96 * j + 96],
                            dsc[:, 128 * j : 128 * (j + 1)],
                            idb[0:96, 0:96],
                        )
                    dst = qkTr[:, 4 * i : 4 * i + 4, :,
                               64 * g : 64 * g + 48]
                    srcv = ptr[:].rearrange(
                        "p (s b c) -> p s b c", s=4, b=2, c=48
                    )
                    nc.vector.tensor_copy(dst, srcv)

                    if g == 1:
                        # gram accumulation; rhs skips the 16-wide pad strips
                        for j in range(4):
                            s = 4 * i + j
                            for h in range(2):
                                lhs = qkT[
                                    :,
                                    256 * s + 128 * h :
                                    256 * s + 128 * h + 112,
                                ]
                                qv = qkT[:]
                                rhs96 = AP(
                                    qv.tensor,
                                    qv.offset + 256 * s + 128 * h,
                                    [tuple(qv.ap[0]), (64, 2), (1, 48)],
                                )
                                nc.tensor.matmul(
                                    grams[h],
                                    lhs,
                                    rhs96,
                                    start=(s == 0), stop=(s == 127),
                                )

            def emitAv(i):
                xt0 = xsp.tile([GCH, CH], BF16, tag="xs0")
                xt1 = xsp.tile([GCH, CH], BF16, tag="xs1")
                nc.sync.dma_start(xt0[:], x_half[0][:, CH * i : CH * (i + 1)])
                nc.sync.dma_start(xt1[:], x_half[1][:, CH * i : CH * (i + 1)])
                praw = ps_raw.tile([128, CH], F32, tag="praw")
                nc.tensor.matmul(
                    praw[:], wv0[:], xt0[:], start=True, stop=False,
                )
                nc.tensor.matmul(
                    praw[:], wv1[:], xt1[:], start=False, stop=True,
                )
                dst = pad[:, 4 * i + 1 : 4 * i + 5, 2:130]
                srcv = praw[:].rearrange("p (r x) -> p r x", r=4, x=128)
                nc.scalar.copy(dst, srcv)

            for i in range(NCHUNK):
                emitA8(0, i)
            for ip in range(NCHUNK // 2):
                emitB8(0, ip)
                emitA8(1, 2 * ip)
                emitA8(1, 2 * ip + 1)
            for ip in range(NCHUNK // 2):
                emitB8(1, ip)
                emitAv(2 * ip)
                emitAv(2 * ip + 1)

            # ---- attention finalize (overlaps v sweep A; heads interleaved
            # so the two serial chains hide each other's latency) ----
            st = {}
            for h in range(2):
                gh = grams[h]
                n2full = sp.tile([112, 96], F32, tag="n2full",
                                 name=f"n2full_{h}")
                nc.vector.tensor_mul(n2full[:], gh, idm[:])
                st[h] = {"n2full": n2full}
            for h in range(2):
                n2 = sp.tile([112, 1], F32, tag="n2", name=f"n2_{h}")
                nc.vector.reduce_sum(
                    n2[:], st[h]["n2full"][:], axis=mybir.AxisListType.X
                )
                st[h]["n2"] = n2
            for h in range(2):
                nrm = sp.tile([112, 1], F32, tag="nrm", name=f"nrm_{h}")
                nc.scalar.sqrt(nrm[:], st[h]["n2"][:])
                nc.vector.tensor_scalar_max(nrm[:], nrm[:], EPS)
                st[h]["nrm"] = nrm
            for h in range(2):
                rr = sp.tile([112, 1], F32, tag="rr", name=f"rr_{h}")
                nc.vector.reciprocal(rr[:], st[h]["nrm"][:])
                st[h]["rr"] = rr
            for h in range(2):
                gh = grams[h]
                rr = st[h]["rr"]
                gkq = sp.tile([48, 48], F32, tag="gkq", name=f"gkq_{h}")
                nc.vector.tensor_copy(gkq[:], gh[64:112, 0:48])
                rk0 = sp.tile([48, 1], F32, tag="rk0", name=f"rk0_{h}")
                nc.vector.tensor_copy(rk0[:], rr[64:112, :])
                askq = sp.tile([48, 48], F32, tag="askq", name=f"askq_{h}")
                nc.vector.tensor_scalar_mul(askq[:], gkq[:], rk0[:])
                st[h]["askq"] = askq
            for h in range(2):
                ptr2 = ps_tr.tile([48, 48], F32, tag="ptr", name=f"ptr2_{h}")
                nc.tensor.transpose(ptr2[:], st[h]["askq"][:], idf[0:48, 0:48])
                st[h]["ptr2"] = ptr2
            for h in range(2):
                gs = sp.tile([48, 48], F32, tag="gs", name=f"gs_{h}")
                nc.vector.tensor_copy(gs[:], st[h]["ptr2"][:])
                sc = sp.tile([48, 1], F32, tag="sc", name=f"sc_{h}")
                nc.vector.tensor_mul(sc[:], st[h]["rr"][0:48, :], tv[:, h : h + 1])
                st[h]["gs"], st[h]["sc"] = gs, sc
            for h in range(2):
                mx = sp.tile([48, 1], F32, tag="mx", name=f"mx_{h}")
                nc.vector.reduce_max(mx[:], st[h]["gs"][:],
                                     axis=mybir.AxisListType.X)
                nbias = sp.tile([48, 1], F32, tag="nb", name=f"nb_{h}")
                nc.vector.tensor_mul(nbias[:], mx[:], st[h]["sc"][:])
                nc.vector.tensor_scalar_mul(nbias[:], nbias[:], -1.0)
                st[h]["nb"] = nbias
            for h in range(2):
                ex = sp.tile([48, 48], F32, tag="ex", name=f"ex_{h}")
                nc.scalar.activation(
                    ex[:], st[h]["gs"][:], mybir.ActivationFunctionType.Exp,
                    bias=st[h]["nb"][:], scale=st[h]["sc"][:],
                )
                st[h]["ex"] = ex
            for h in range(2):
                sm = sp.tile([48, 1], F32, tag="sm", name=f"sm_{h}")
                nc.vector.reduce_sum(sm[:], st[h]["ex"][:],
                                     axis=mybir.AxisListType.X)
                rs = sp.tile([48, 1], F32, tag="rs", name=f"rs_{h}")
                nc.vector.reciprocal(rs[:], sm[:])
                ab = sp.tile([48, 48], BF16, tag="ab", name=f"ab_{h}")
                nc.vector.tensor_scalar_mul(ab[:], st[h]["ex"][:], rs[:])
                st[h]["ab"] = ab
            for h in range(2):
                pmt = ps_tr.tile([48, DIM], F32, tag="ptr", name=f"pmt_{h}")
                nc.tensor.matmul(
                    pmt[:], st[h]["ab"][:], wph[h][:], start=True, stop=True,
                )
                st[h]["pmt"] = pmt
            for h in range(2):
                nc.vector.tensor_copy(mt[64 * h : 64 * h + 48, :],
                                      st[h]["pmt"][:])

            # ---- sweep B: depthwise 3x3; PE diag matmuls + DVE taps ----
            for ip in range(NCHUNK // 2):
                def tap_rhs(t, i):
                    dy, dx = divmod(t, 3)
                    dy -= 1
                    dx -= 1
                    return pad[:, 4 * i + 1 + dy : 4 * i + 5 + dy,
                               2 + dx : 130 + dx]

                pdws = [
                    ps_dw.tile([128, CH], F32, tag="pdw", name=f"pdwv_{i}")
                    for i in (2 * ip, 2 * ip + 1)
                ]
                for n_, t in enumerate(PE_TAPS_V):
                    for k, i in enumerate((2 * ip, 2 * ip + 1)):
                        nc.tensor.matmul(
                            pdws[k][:],
                            wd[:, t * 128 : t * 128 + 128],
                            tap_rhs(t, i),
                            start=(n_ == 0),
                            stop=(n_ == len(PE_TAPS_V) - 1),
                        )

                acc = sp.tile([128, 8 * RS], BF16, tag="acc")
                for k2 in range(2):
                    base0 = (4 * (2 * ip + k2) + 1) * RS + 2
                    ah = acc[:, 4 * RS * k2 : 4 * RS * (k2 + 1)]
                    first = True
                    for t in DVE_TAPS_V:
                        dy, dx = divmod(t, 3)
                        off = base0 + (dy - 1) * RS + (dx - 1)
                        fl = padf[:, off : off + 4 * RS]
                        wcol = wt[:, t : t + 1]
                        if first:
                            nc.vector.tensor_scalar_mul(ah, fl, wcol)
                            first = False
                        else:
                            nc.vector.scalar_tensor_tensor(
                                ah, fl, wcol, ah,
                                op0=mybir.AluOpType.mult,
                                op1=mybir.AluOpType.add,
                            )

                accvs = [
                    acc[:, 4 * RS * k : 4 * RS * (k + 1)].rearrange(
                        "p (r x) -> p r x", r=4, x=RS
                    )[:, :, 0:128]
                    for k in range(2)
                ]

                for k, i in enumerate((2 * ip, 2 * ip + 1)):
                    vsc = sp.tile([128, CH], BF16, tag="vsc",
                                  name=f"vsc_{i}")
                    dstv = vsc[:].rearrange("p (r x) -> p r x", r=4, x=128)
                    pdwv = pdws[k][:].rearrange("p (r x) -> p r x", r=4, x=128)
                    nc.vector.scalar_tensor_tensor(
                        dstv, pdwv, 1.0, accvs[k],
                        op0=mybir.AluOpType.mult,
                        op1=mybir.AluOpType.add,
                    )
                    for mj in range(2):
                        pout = ps_tr.tile([GCH, CH], F32, tag="ptr",
                                          name=f"pout_{i}_{mj}")
                        nc.tensor.matmul(
                            pout[:], mt[:, 96 * mj : 96 * mj + 96],
                            vsc[:],
                            start=True, stop=True,
                        )
                        ost = stp.tile([GCH, CH], BF16, tag="ost",
                                       name=f"ost_{i}_{mj}")
                        nc.scalar.copy(ost[:], pout[:])
                        nc.sync.dma_start(
                            outp[96 * mj : 96 * mj + 96,
                                 CH * i : CH * (i + 1)],
                            ost[:],
                        )

    return nc


_NC_CACHE = None


def _get_nc(split=True):
    global _NC_CACHE
    if _NC_CACHE is None:
        _NC_CACHE = _build_kernel()
        if split:
            # needed for walrus codegen in this env; breaks CoreSim, so only
            # applied on the hardware path
            _split_multiwait(_NC_CACHE)
    return _NC_CACHE


def _swi_pack(A, Bm):
    """SwInterleave weight packing: out[k, 2j] = A[k, M-1-j],
    out[k, 2j+1] = B[k, M-1-j]."""
    K, M = A.shape
    out = np.zeros((K, 2 * M), dtype=np.float32)
    out[:, 0::2] = A[:, ::-1]
    out[:, 1::2] = Bm[:, ::-1]
    return out


def make_in_maps(x, w_qkv, w_dw, w_proj, temperature):
    x = np.asarray(x, dtype=np.float32)
    w_qkv = np.asarray(w_qkv, dtype=np.float32)
    w_dw = np.asarray(w_dw, dtype=np.float32).reshape(3 * DIM, 3, 3)
    w_proj = np.asarray(w_proj, dtype=np.float32)
    temperature = np.asarray(temperature, dtype=np.float32).reshape(HEADS)
    bf = ml_dtypes.bfloat16
    f8 = ml_dtypes.float8_e4m3fn

    in_maps = []
    for m in range(8):
        b, p = divmod(m, 2)
        rows = np.concatenate(
            [np.arange(96 * p + off, 96 * p + off + 96) for off in (0, DIM, 2 * DIM)]
        )  # q(96), k(96), v(96) global rows in w_qkv / w_dw
        wq = w_qkv[rows, :]                      # [288, 192] (q, k, v)
        dw = w_dw[rows]                          # [288, 3, 3]

        # v-group wqT columns, heads at +0 and +64 (128-padded, v path only)
        wqT = np.zeros((DIM, 384), dtype=np.float32)
        for g in range(3):
            wqT[:, 128 * g : 128 * g + 48] = wq[96 * g : 96 * g + 48].T
            wqT[:, 128 * g + 64 : 128 * g + 112] = wq[96 * g + 48 : 96 * g + 96].T

        # qk GEMM weights fp8 SwInterleaved, FD96 (heads contiguous):
        # per group, A = K-half 0:96, B = K-half 96:192
        wq8 = np.zeros((GCH, 512), dtype=np.float32)
        for g in range(2):
            wg = np.zeros((DIM, 128), dtype=np.float32)
            wg[:, 0:96] = wq[96 * g : 96 * g + 96].T
            wq8[:, 256 * g : 256 * g + 256] = _swi_pack(wg[0:96], wg[96:192])

        # qk dwconv DR pair weights fp8
        def diag_vec(g, t):
            d = np.zeros(96, dtype=np.float32)
            if t is not None:
                d[:] = dw[96 * g : 96 * g + 96, t // 3, t % 3]
            return d

        wd8 = np.zeros((GCH, 2 * 5 * 256), dtype=np.float32)
        for g in range(2):
            for j, (ta, tb, _offa, _dlt) in enumerate(DR_PAIRS):
                da = diag_vec(g, ta)
                db = diag_vec(g, tb)
                blk = wd8[:, (5 * g + j) * 256 : (5 * g + j + 1) * 256]
                kk = np.arange(96)
                blk[kk, 2 * (127 - kk)] = da
                blk[kk, 2 * (127 - kk) + 1] = db

        # v dwconv diag weights bf16: [128, 9*128]
        wdiag = np.zeros((128, 9 * 128), dtype=np.float32)
        for t in range(9):
            d = dw[192:288, t // 3, t % 3]
            blk = wdiag[:, t * 128 : (t + 1) * 128]
            np.fill_diagonal(blk[0:48, 0:48], d[0:48])
            np.fill_diagonal(blk[64:112, 64:112], d[48:96])

        wpT = np.ascontiguousarray(w_proj[:, 96 * p : 96 * p + 96].T)  # [96, 192]
        wtapm = np.zeros((128, 9), dtype=np.float32)
        for t in range(9):
            d = dw[192:288, t // 3, t % 3]
            wtapm[0:48, t] = d[0:48]
            wtapm[64:112, t] = d[48:96]
        tempvm = np.empty((48, 2), dtype=np.float32)
        tempvm[:, 0] = temperature[2 * p]
        tempvm[:, 1] = temperature[2 * p + 1]
        xb = x[b].reshape(DIM, NPIX)
        x8 = np.concatenate([xb[0:96], xb[96:192]], axis=1)  # [96, 32768]
        in_maps.append(
            {
                "x8": x8.astype(f8),
                "x0": xb[:96].astype(bf),
                "x1": xb[96:].astype(bf),
                "wq8": wq8.astype(f8),
                "wqv0": wqT[0:96, 256:384].astype(bf),
                "wqv1": wqT[96:192, 256:384].astype(bf),
                "wd8": wd8.astype(f8),
                "wdiag": wdiag.astype(bf),
                "wpT": wpT.astype(bf),
                "wtap": wtapm,
                "tempv": tempvm,
                "id128": np.eye(128, dtype=np.float32),
            }
        )
    return in_maps


def kernel(x, w_qkv, w_dw, w_proj, temperature):
    nc = _get_nc()
    in_maps = make_in_maps(x, w_qkv, w_dw, w_proj, temperature)
    res = run_bass_kernel_spmd(nc, in_maps, core_ids=list(range(8)))
    out = np.empty((B, DIM, HH, WW), dtype=np.float32)
    for b in range(B):
        part = (
            res.results[2 * b]["outp"].astype(np.float32)
            + res.results[2 * b + 1]["outp"].astype(np.float32)
        )
        out[b] = part.reshape(DIM, HH, WW)
    return out


# revision 20
# speedup vs baseline: 1.1710x; 1.1710x over previous
"""Trainium2 Bass kernel for channel attention (XCA-style) module.

Computation (per batch b):
  qkv = w_qkv @ x          (1x1 conv, 192 -> 576 ch)
  qkv = dwconv3x3(qkv)     (depthwise, pad 1)
  q,k,v = split; per head (48 ch): l2-normalize q,k along spatial,
  attn = softmax(temp * q_hat k_hat^T); out = attn @ v
  out = w_proj @ out       (1x1 conv)

Sharding: 8 cores = 4 batches x 2 head-pairs. Each core handles one batch and
2 of the 4 heads (288 of 576 qkv channels), producing a partial projection
output [192, 16384]; host sums the two partials per batch.

q/k path runs in fp8 (e4m3) with DoubleRowSwInterleave matmuls: the 1x1 GEMM
folds its two 96-channel K-halves into one DR matmul per chunk, and the 3x3
depthwise conv runs as 5 DR tap-pair matmuls per chunk against a gapless
(row stride 128) fp8 pad buffer. The gapless layout makes dx=+-1 taps wrap
2 edge columns into the neighboring row; the resulting q/k noise is tiny and
washes out in the 16384-wide gram contraction. v path stays bf16 for accuracy
(its error passes straight to the output).

Layouts (per core):
  q/k groups: 96 channels (h0 0:48, h1 64:112) on a 128-padded partition dim
  v group: same 128-partition padding
  qkT store: per 128-pixel subtile s, columns [s*256, s*256+256) =
             [q_h0(48) pad k_h0(48) pad q_h1(48) pad k_h1(48) pad]
  gram per head: [112, 112] from 64-padded stacked [q_h | k_h] blocks.
"""

import sys

sys.path.insert(0, "/opt/trn_rl_repo")

import numpy as np
import ml_dtypes

import concourse.bass as bass
import concourse.mybir as mybir
from concourse import tile
from concourse.bass_types import AP
from concourse.bass_utils import run_bass_kernel_spmd

F32 = mybir.dt.float32
BF16 = mybir.dt.bfloat16
F8 = mybir.dt.float8e4
SWI = mybir.MatmulPerfMode.DoubleRowSwInterleave

DIM = 192
HEADS = 4
B = 4
HH = 128
WW = 128
NPIX = HH * WW          # 16384
GCH = 96                # q/k channels per core (2 heads x 48)
NCHUNK = 32             # 512-pixel chunks
CH = 512                # chunk size (4 image rows)
EPS = 1e-12
RS = 132                # v pad row stride (2 pad cols each side)
G8 = 128                # fp8 pad guard prefix/suffix (elements)
# fp8 pad: flat(pr, col) = G8 + pr*128 + col, pr 0..129 (pr0/pr129 zero rows)
PAD8_LEN = G8 + 130 * 128 + G8

# DR tap pairs for the fp8 dwconv: (tA, tB, offA, delta); tB None = dummy
# off(t) = dy*128 + dx for t = 3*(dy+1) + (dx+1)
DR_PAIRS = [
    (0, 2, -129, 2),
    (3, 5, -1, 2),
    (6, 8, 127, 2),
    (1, 7, -128, 256),
    (4, None, 0, 2),
]
# v-group tap split (RS=132 padded bf16 buffer)
PE_TAPS_V = [0, 2, 3, 5, 6, 7, 8]
DVE_TAPS_V = [1, 4]
PE_TAPS_V_EVEN = [0, 2, 3, 5, 6, 8]
DVE_TAPS_V_EVEN = [1, 4, 7]


def _split_multiwait(nc):
    """walrus in this env only encodes one sem-wait per instruction; hoist
    extra waits into single-wait NoOps placed just before the instruction."""
    for f in nc.m.functions:
        for bb in f.blocks:
            insts = bb.instructions
            i = 0
            while i < len(insts):
                inst = insts[i]
                si = getattr(inst, "sync_info", None)
                ow = list(si.on_wait) if (si is not None and si.on_wait) else []
                if len(ow) > 1:
                    nops = []
                    for w in ow[:-1]:
                        nops.append(
                            mybir.InstNoOp(
                                name=nc.get_next_instruction_name(),
                                sync_info=mybir.SyncInfo(on_wait=[w], on_update=[]),
                                bass_nofuse=True,
                                engine=inst.engine,
                            )
                        )
                    inst.sync_info = mybir.SyncInfo(
                        on_wait=[ow[-1]], on_update=list(si.on_update)
                    )
                    insts[i:i] = nops
                    i += len(nops)
                i += 1


def _build_kernel():
    nc = bass.Bass("TRN2", target_bir_lowering=False, debug=False, num_devices=8)

    # ---- DRAM I/O ----
    # qk path: fp8 x, both halves side by side [96, 2*16384]
    x8 = nc.dram_tensor("x8", [GCH, 2 * NPIX], F8, kind="ExternalInput")
    # v path: bf16 x halves
    x_half = [
        nc.dram_tensor(f"x{h}", [GCH, NPIX], BF16, kind="ExternalInput")
        for h in range(2)
    ]
    # qk GEMM weights, SwInterleaved: [96, 2 groups * 256]
    wq8 = nc.dram_tensor("wq8", [GCH, 512], F8, kind="ExternalInput")
    # v GEMM weights bf16: wqT v-columns only
    wqv0 = nc.dram_tensor("wqv0", [GCH, 128], BF16, kind="ExternalInput")
    wqv1 = nc.dram_tensor("wqv1", [GCH, 128], BF16, kind="ExternalInput")
    # qk dwconv DR pair weights: [128, 2 groups * 5 pairs * 256]
    wd8 = nc.dram_tensor("wd8", [GCH, 2 * 5 * 256], F8, kind="ExternalInput")
    # v dwconv diag weights bf16: [128, 9*128]
    wdiag = nc.dram_tensor("wdiag", [128, 9 * 128], BF16, kind="ExternalInput")
    wpT = nc.dram_tensor("wpT", [GCH, DIM], BF16, kind="ExternalInput")
    wtap = nc.dram_tensor("wtap", [128, 9], F32, kind="ExternalInput")
    tempv = nc.dram_tensor("tempv", [48, 2], F32, kind="ExternalInput")
    id128 = nc.dram_tensor("id128", [128, 128], F32, kind="ExternalInput")
    outp = nc.dram_tensor("outp", [DIM, NPIX], BF16, kind="ExternalOutput")

    with tile.TileContext(nc) as tc:
        with (
            tc.tile_pool(name="persist", bufs=1) as pp,
            tc.tile_pool(name="scratch", bufs=6) as sp,
            tc.tile_pool(name="stage", bufs=4) as stp,
            tc.tile_pool(name="xstream", bufs=8) as xsp,
            tc.tile_pool(name="ps_raw", bufs=3, space="PSUM") as ps_raw,
            tc.tile_pool(name="ps_dw", bufs=2, space="PSUM") as ps_dw,
            tc.tile_pool(name="ps_tr", bufs=2, space="PSUM") as ps_tr,
            tc.tile_pool(name="ps_gram", bufs=1, space="PSUM") as ps_gram,
        ):
            # ---- persistent SBUF ----
            # first-needed weights on the sync queue; rest via Pool soft-DGE
            wq8t = pp.tile([GCH, 512], F8, tag="wq8t")
            nc.sync.dma_start(wq8t[:], wq8[:])
            wd8t = pp.tile([GCH, 2 * 5 * 256], F8, tag="wd8t")
            nc.gpsimd.dma_start(wd8t[:], wd8[:])
            wv0 = pp.tile([GCH, 128], BF16, tag="wv0")
            nc.gpsimd.dma_start(wv0[:], wqv0[:])
            wv1 = pp.tile([GCH, 128], BF16, tag="wv1")
            nc.gpsimd.dma_start(wv1[:], wqv1[:])
            wd = pp.tile([128, 9 * 128], BF16, tag="wd")
            nc.gpsimd.dma_start(wd[:], wdiag[:])
            wph = []
            for h in range(2):
                t = pp.tile([48, DIM], BF16, tag=f"wp{h}", name=f"wpt{h}")
                nc.gpsimd.dma_start(t[:], wpT[48 * h : 48 * h + 48, :])
                wph.append(t)
            wt = pp.tile([128, 9], F32, tag="wt")
            nc.gpsimd.dma_start(wt[:], wtap[:])
            tv = pp.tile([48, 2], F32, tag="tv")
            nc.gpsimd.dma_start(tv[:], tempv[:])
            idf = pp.tile([128, 128], F32, tag="idf")
            nc.gpsimd.dma_start(idf[:], id128[:])
            idb = pp.tile([128, 128], BF16, tag="idb")
            nc.vector.tensor_copy(idb[:], idf[:])

            # preload the sqrt/exp activation table sets during the startup
            # window so no ACT_TABLE_LOAD fires in the serial k->v finalize
            dum = pp.tile([1, 4], F32, tag="dum")
            nc.vector.memset(dum[:], 1.0)
            nc.scalar.sqrt(dum[:, 2:3], dum[:, 0:1])
            nc.scalar.activation(
                dum[:, 3:4], dum[:, 0:1], mybir.ActivationFunctionType.Exp
            )

            # fp8 pad buffers for q and k groups: gapless rows (stride 128),
            # pr0/pr129 zero rows, 128-elem zero guards both ends
            pad8s = []
            for pi in range(2):
                p8 = pp.tile([GCH, PAD8_LEN], F8, tag=f"pad8_{pi}",
                             name=f"pad8_{pi}")
                nc.vector.memset(p8[:, 0 : G8 + 128], 0.0)
                nc.vector.memset(p8[:, G8 + 129 * 128 :], 0.0)
                pad8s.append(p8)

            # bf16 padded image buffer for v: 130 rows x 132 cols + tail
            padf = pp.tile([128, 130 * RS + 4], BF16, tag="padv")
            pad = padf[:, 0 : 130 * RS].rearrange("p (r x) -> p r x", r=130, x=RS)
            nc.vector.memset(pad[:, 0, :], 0.0)
            nc.vector.memset(pad[:, 129, :], 0.0)
            nc.vector.memset(pad[:, :, 0:2], 0.0)
            nc.vector.memset(pad[:, :, 130:132], 0.0)
            nc.vector.memset(padf[:, 130 * RS :], 0.0)

            # transposed q/k store: per 128-pixel subtile s, 256 columns:
            # [q_h0(48) pad(16) k_h0(48) pad(16) q_h1(48) pad(16) k_h1(48) pad(16)]
            qkT = pp.tile([128, 128 * 256], BF16, tag="qkT")
            qkTr = qkT[:].rearrange("p (s b c) -> p s b c", s=128, b=2, c=128)
            qkTp = qkT[:].rearrange("p (s b c) -> p s b c", s=128, b=4, c=64)
            nc.gpsimd.memset(qkTp[:, :, :, 48:64], 0.0)

            # gram accumulators: both heads share one PSUM bank; FD=112
            gramt = ps_gram.tile([112, 224], F32, tag="gram")
            grams = [gramt[:, 112 * h : 112 * (h + 1)] for h in range(2)]

            # M^T for final GEMM: [128(d-padded), 192(o)] bf16
            mt = pp.tile([128, DIM], BF16, tag="mt")
            nc.gpsimd.memset(mt[:], 0.0)

            x8r = x8[:].rearrange("p (h n) -> p h n", h=2)

            # ================= fused sweep emission =================
            # A(g): 1x1 GEMM chunk into pad buffer; B(g): dwconv+downstream.
            # Emission order A(q); B(q)+A(k); B(k)+A(v); finalize; B(v) keeps
            # the PE busy with taps while ACT fills the next group's pad.
            def emitA8(g, i, dq=None):
                p8 = pad8s[g]
                xt8 = xsp.tile([GCH, 1024], F8, tag="xs8", name=f"x8_{g}_{i}")
                (dq or nc.sync).dma_start(
                    xt8[:].rearrange("p (h n) -> p h n", h=2),
                    x8r[:, :, CH * i : CH * (i + 1)],
                )
                praw = ps_raw.tile([128, CH], F32, tag="praw",
                                   name=f"praw8_{g}_{i}")
                nc.tensor.matmul(
                    praw[:],
                    wq8t[:, 256 * g : 256 * g + 256],
                    xt8[:].rearrange("p (h n) -> p h n", h=2),
                    start=True, stop=True, perf_mode=SWI,
                )
                base = G8 + (4 * i + 1) * 128
                nc.scalar.copy(p8[:, base : base + 512], praw[0:GCH, :])

            def emitB8(g, ip):
                p8v = pad8s[g][:]
                pdws = [
                    ps_dw.tile([128, CH], F32, tag="pdw", name=f"pdw_{g}_{i}")
                    for i in (2 * ip, 2 * ip + 1)
                ]
                # pair-outer so consecutive matmuls share stationary
                for j, (ta, tb, offa, dlt) in enumerate(DR_PAIRS):
                    for k, i in enumerate((2 * ip, 2 * ip + 1)):
                        base = G8 + (4 * i + 1) * 128
                        rhs = AP(
                            p8v.tensor,
                            p8v.offset + base + offa,
                            [tuple(p8v.ap[0]), (dlt, 2), (1, 512)],
                        )
                        nc.tensor.matmul(
                            pdws[k][:],
                            wd8t[:, (5 * g + j) * 256 : (5 * g + j + 1) * 256],
                            rhs,
                            start=(j == 0), stop=(j == 4),
                            perf_mode=SWI,
                        )

                for k, i in enumerate((2 * ip, 2 * ip + 1)):
                    # evac bf16, transpose 4x blocks, store into qkT slots
                    dsc = sp.tile([GCH, CH], BF16, tag="dsc",
                                  name=f"dsc_{g}_{i}")
                    nc.vector.tensor_copy(dsc[:], pdws[k][0:GCH, :])
                    ptr = ps_tr.tile([128, 4 * GCH], BF16, tag="ptr",
                                     name=f"ptr_{g}_{i}")
                    for j in range(4):
                        nc.tensor.transpose(
                            ptr[:, 96 * j : 96 * j + 96],
                            dsc[:, 128 * j : 128 * (j + 1)],
                            idb[0:96, 0:96],
                        )
                    dst = qkTr[:, 4 * i : 4 * i + 4, :,
                               64 * g : 64 * g + 48]
                    srcv = ptr[:].rearrange(
                        "p (s b c) -> p s b c", s=4, b=2, c=48
                    )
                    nc.vector.tensor_copy(dst, srcv)

                    if g == 1:
                        # gram accumulation for both heads
                        for j in range(4):
                            s = 4 * i + j
                            for h in range(2):
                                lhs = qkT[
                                    :,
                                    256 * s + 128 * h :
                                    256 * s + 128 * h + 112,
                                ]
                                nc.tensor.matmul(
                                    grams[h],
                                    lhs,
                                    lhs,
                                    start=(s == 0), stop=(s == 127),
                                )

            def emitAv(i):
                xt0 = xsp.tile([GCH, CH], BF16, tag="xs0")
                xt1 = xsp.tile([GCH, CH], BF16, tag="xs1")
                nc.sync.dma_start(xt0[:], x_half[0][:, CH * i : CH * (i + 1)])
                nc.sync.dma_start(xt1[:], x_half[1][:, CH * i : CH * (i + 1)])
                praw = ps_raw.tile([128, CH], F32, tag="praw")
                nc.tensor.matmul(
                    praw[:], wv0[:], xt0[:], start=True, stop=False,
                )
                nc.tensor.matmul(
                    praw[:], wv1[:], xt1[:], start=False, stop=True,
                )
                dst = pad[:, 4 * i + 1 : 4 * i + 5, 2:130]
                srcv = praw[:].rearrange("p (r x) -> p r x", r=4, x=128)
                nc.scalar.copy(dst, srcv)

            for i in range(NCHUNK):
                emitA8(0, i)
            for ip in range(NCHUNK // 2):
                emitB8(0, ip)
                emitA8(1, 2 * ip)
                emitA8(1, 2 * ip + 1)
            for ip in range(NCHUNK // 2):
                emitB8(1, ip)
                emitAv(2 * ip)
                emitAv(2 * ip + 1)

            # ---- attention finalize (overlaps v sweep A; heads interleaved
            # so the two serial chains hide each other's latency) ----
            st = {}
            for h in range(2):
                gh = grams[h]
                n2full = sp.tile([112, 96], F32, tag="n2full",
                                 name=f"n2full_{h}")
                nc.vector.tensor_mul(n2full[:], gh, idm[:])
                st[h] = {"n2full": n2full}
            for h in range(2):
                n2 = sp.tile([112, 1], F32, tag="n2", name=f"n2_{h}")
                nc.vector.reduce_sum(
                    n2[:], st[h]["n2full"][:], axis=mybir.AxisListType.X
                )
                st[h]["n2"] = n2
            for h in range(2):
                nrm = sp.tile([112, 1], F32, tag="nrm", name=f"nrm_{h}")
                nc.scalar.sqrt(nrm[:], st[h]["n2"][:])
                nc.vector.tensor_scalar_max(nrm[:], nrm[:], EPS)
                st[h]["nrm"] = nrm
            for h in range(2):
                rr = sp.tile([112, 1], F32, tag="rr", name=f"rr_{h}")
                nc.vector.reciprocal(rr[:], st[h]["nrm"][:])
                st[h]["rr"] = rr
            for h in range(2):
                gh = grams[h]
                rr = st[h]["rr"]
                gkq = sp.tile([48, 48], F32, tag="gkq", name=f"gkq_{h}")
                nc.vector.tensor_copy(gkq[:], gh[64:112, 0:48])
                rk0 = sp.tile([48, 1], F32, tag="rk0", name=f"rk0_{h}")
                nc.vector.tensor_copy(rk0[:], rr[64:112, :])
                askq = sp.tile([48, 48], F32, tag="askq", name=f"askq_{h}")
                nc.vector.tensor_scalar_mul(askq[:], gkq[:], rk0[:])
                st[h]["askq"] = askq
            for h in range(2):
                ptr2 = ps_tr.tile([48, 48], F32, tag="ptr", name=f"ptr2_{h}")
                nc.tensor.transpose(ptr2[:], st[h]["askq"][:], idf[0:48, 0:48])
                st[h]["ptr2"] = ptr2
            for h in range(2):
                gs = sp.tile([48, 48], F32, tag="gs", name=f"gs_{h}")
                nc.vector.tensor_copy(gs[:], st[h]["ptr2"][:])
                sc = sp.tile([48, 1], F32, tag="sc", name=f"sc_{h}")
                nc.vector.tensor_mul(sc[:], st[h]["rr"][0:48, :], tv[:, h : h + 1])
                st[h]["gs"], st[h]["sc"] = gs, sc
            for h in range(2):
                mx = sp.tile([48, 1], F32, tag="mx", name=f"mx_{h}")
                nc.vector.reduce_max(mx[:], st[h]["gs"][:],
                                     axis=mybir.AxisListType.X)
                nbias = sp.tile([48, 1], F32, tag="nb", name=f"nb_{h}")
                nc.vector.tensor_mul(nbias[:], mx[:], st[h]["sc"][:])
                nc.vector.tensor_scalar_mul(nbias[:], nbias[:], -1.0)
                st[h]["nb"] = nbias
            for h in range(2):
                ex = sp.tile([48, 48], F32, tag="ex", name=f"ex_{h}")
                nc.scalar.activation(
                    ex[:], st[h]["gs"][:], mybir.ActivationFunctionType.Exp,
                    bias=st[h]["nb"][:], scale=st[h]["sc"][:],
                )
                st[h]["ex"] = ex
            for h in range(2):
                sm = sp.tile([48, 1], F32, tag="sm", name=f"sm_{h}")
                nc.vector.reduce_sum(sm[:], st[h]["ex"][:],
                                     axis=mybir.AxisListType.X)
                rs = sp.tile([48, 1], F32, tag="rs", name=f"rs_{h}")
                nc.vector.reciprocal(rs[:], sm[:])
                ab = sp.tile([48, 48], BF16, tag="ab", name=f"ab_{h}")
                nc.vector.tensor_scalar_mul(ab[:], st[h]["ex"][:], rs[:])
                st[h]["ab"] = ab
            for h in range(2):
                pmt = ps_tr.tile([48, DIM], F32, tag="ptr", name=f"pmt_{h}")
                nc.tensor.matmul(
                    pmt[:], st[h]["ab"][:], wph[h][:], start=True, stop=True,
                )
                st[h]["pmt"] = pmt
            for h in range(2):
                nc.vector.tensor_copy(mt[64 * h : 64 * h + 48, :],
                                      st[h]["pmt"][:])

            # ---- sweep B: depthwise 3x3; PE diag matmuls + DVE taps ----
            for ip in range(NCHUNK // 2):
                def tap_rhs(t, i):
                    dy, dx = divmod(t, 3)
                    dy -= 1
                    dx -= 1
                    return pad[:, 4 * i + 1 + dy : 4 * i + 5 + dy,
                               2 + dx : 130 + dx]

                pdws = [
                    ps_dw.tile([128, CH], F32, tag="pdw", name=f"pdwv_{i}")
                    for i in (2 * ip, 2 * ip + 1)
                ]
                for n_, t in enumerate(PE_TAPS_V):
                    for k, i in enumerate((2 * ip, 2 * ip + 1)):
                        nc.tensor.matmul(
                            pdws[k][:],
                            wd[:, t * 128 : t * 128 + 128],
                            tap_rhs(t, i),
                            start=(n_ == 0),
                            stop=(n_ == len(PE_TAPS_V) - 1),
                        )

                acc = sp.tile([128, 8 * RS], BF16, tag="acc")
                for k2 in range(2):
                    base0 = (4 * (2 * ip + k2) + 1) * RS + 2
                    ah = acc[:, 4 * RS * k2 : 4 * RS * (k2 + 1)]
                    first = True
                    for t in DVE_TAPS_V:
                        dy, dx = divmod(t, 3)
                        off = base0 + (dy - 1) * RS + (dx - 1)
                        fl = padf[:, off : off + 4 * RS]
                        wcol = wt[:, t : t + 1]
                        if first:
                            nc.vector.tensor_scalar_mul(ah, fl, wcol)
                            first = False
                        else:
                            nc.vector.scalar_tensor_tensor(
                                ah, fl, wcol, ah,
                                op0=mybir.AluOpType.mult,
                                op1=mybir.AluOpType.add,
                            )

                accvs = [
                    acc[:, 4 * RS * k : 4 * RS * (k + 1)].rearrange(
                        "p (r x) -> p r x", r=4, x=RS
                    )[:, :, 0:128]
                    for k in range(2)
                ]

                for k, i in enumerate((2 * ip, 2 * ip + 1)):
                    vsc = sp.tile([128, CH], BF16, tag="vsc",
                                  name=f"vsc_{i}")
                    dstv = vsc[:].rearrange("p (r x) -> p r x", r=4, x=128)
                    pdwv = pdws[k][:].rearrange("p (r x) -> p r x", r=4, x=128)
                    nc.vector.scalar_tensor_tensor(
                        dstv, pdwv, 1.0, accvs[k],
                        op0=mybir.AluOpType.mult,
                        op1=mybir.AluOpType.add,
                    )
                    for mj in range(2):
                        pout = ps_tr.tile([GCH, CH], F32, tag="ptr",
                                          name=f"pout_{i}_{mj}")
                        nc.tensor.matmul(
                            pout[:], mt[:, 96 * mj : 96 * mj + 96],
                            vsc[:],
                            start=True, stop=True,
                        )
                        ost = stp.tile([GCH, CH], BF16, tag="ost",
                                       name=f"ost_{i}_{mj}")
                        nc.scalar.copy(ost[:], pout[:])
                        nc.sync.dma_start(
                            outp[96 * mj : 96 * mj + 96,
                                 CH * i : CH * (i + 1)],
                            ost[:],
                        )

    return nc


_NC_CACHE = None


def _get_nc(split=True):
    global _NC_CACHE
    if _NC_CACHE is None:
        _NC_CACHE = _build_kernel()
        if split:
            # needed for walrus codegen in this env; breaks CoreSim, so only
            # applied on the hardware path
            _split_multiwait(_NC_CACHE)
    return _NC_CACHE


def _swi_pack(A, Bm):
    """SwInterleave weight packing: out[k, 2j] = A[k, M-1-j],
    out[k, 2j+1] = B[k, M-1-j]."""
    K, M = A.shape
    out = np.zeros((K, 2 * M), dtype=np.float32)
    out[:, 0::2] = A[:, ::-1]
    out[:, 1::2] = Bm[:, ::-1]
    return out


def make_in_maps(x, w_qkv, w_dw, w_proj, temperature):
    x = np.asarray(x, dtype=np.float32)
    w_qkv = np.asarray(w_qkv, dtype=np.float32)
    w_dw = np.asarray(w_dw, dtype=np.float32).reshape(3 * DIM, 3, 3)
    w_proj = np.asarray(w_proj, dtype=np.float32)
    temperature = np.asarray(temperature, dtype=np.float32).reshape(HEADS)
    bf = ml_dtypes.bfloat16
    f8 = ml_dtypes.float8_e4m3fn

    in_maps = []
    for m in range(8):
        b, p = divmod(m, 2)
        rows = np.concatenate(
            [np.arange(96 * p + off, 96 * p + off + 96) for off in (0, DIM, 2 * DIM)]
        )  # q(96), k(96), v(96) global rows in w_qkv / w_dw
        wq = w_qkv[rows, :]                      # [288, 192] (q, k, v)
        dw = w_dw[rows]                          # [288, 3, 3]

        # v-group wqT columns, heads at +0 and +64 (128-padded, v path only)
        wqT = np.zeros((DIM, 384), dtype=np.float32)
        for g in range(3):
            wqT[:, 128 * g : 128 * g + 48] = wq[96 * g : 96 * g + 48].T
            wqT[:, 128 * g + 64 : 128 * g + 112] = wq[96 * g + 48 : 96 * g + 96].T

        # qk GEMM weights fp8 SwInterleaved, FD96 (heads contiguous):
        # per group, A = K-half 0:96, B = K-half 96:192
        wq8 = np.zeros((GCH, 512), dtype=np.float32)
        for g in range(2):
            wg = np.zeros((DIM, 128), dtype=np.float32)
            wg[:, 0:96] = wq[96 * g : 96 * g + 96].T
            wq8[:, 256 * g : 256 * g + 256] = _swi_pack(wg[0:96], wg[96:192])

        # qk dwconv DR pair weights fp8
        def diag_vec(g, t):
            d = np.zeros(96, dtype=np.float32)
            if t is not None:
                d[:] = dw[96 * g : 96 * g + 96, t // 3, t % 3]
            return d

        wd8 = np.zeros((GCH, 2 * 5 * 256), dtype=np.float32)
        for g in range(2):
            for j, (ta, tb, _offa, _dlt) in enumerate(DR_PAIRS):
                da = diag_vec(g, ta)
                db = diag_vec(g, tb)
                blk = wd8[:, (5 * g + j) * 256 : (5 * g + j + 1) * 256]
                kk = np.arange(96)
                blk[kk, 2 * (127 - kk)] = da
                blk[kk, 2 * (127 - kk) + 1] = db

        # v dwconv diag weights bf16: [128, 9*128]
        wdiag = np.zeros((128, 9 * 128), dtype=np.float32)
        for t in range(9):
            d = dw[192:288, t // 3, t % 3]
            blk = wdiag[:, t * 128 : (t + 1) * 128]
            np.fill_diagonal(blk[0:48, 0:48], d[0:48])
            np.fill_diagonal(blk[64:112, 64:112], d[48:96])

        wpT = np.ascontiguousarray(w_proj[:, 96 * p : 96 * p + 96].T)  # [96, 192]
        wtapm = np.zeros((128, 9), dtype=np.float32)
        for t in range(9):
            d = dw[192:288, t // 3, t % 3]
            wtapm[0:48, t] = d[0:48]
            wtapm[64:112, t] = d[48:96]
        tempvm = np.empty((48, 2), dtype=np.float32)
        tempvm[:, 0] = temperature[2 * p]
        tempvm[:, 1] = temperature[2 * p + 1]
        xb = x[b].reshape(DIM, NPIX)
        x8 = np.concatenate([xb[0:96], xb[96:192]], axis=1)  # [96, 32768]
        in_maps.append(
            {
                "x8": x8.astype(f8),
                "x0": xb[:96].astype(bf),
                "x1": xb[96:].astype(bf),
                "wq8": wq8.astype(f8),
                "wqv0": wqT[0:96, 256:384].astype(bf),
                "wqv1": wqT[96:192, 256:384].astype(bf),
                "wd8": wd8.astype(f8),
                "wdiag": wdiag.astype(bf),
                "wpT": wpT.astype(bf),
                "wtap": wtapm,
                "tempv": tempvm,
                "id128": np.eye(128, dtype=np.float32),
            }
        )
    return in_maps


def kernel(x, w_qkv, w_dw, w_proj, temperature):
    nc = _get_nc()
    in_maps = make_in_maps(x, w_qkv, w_dw, w_proj, temperature)
    res = run_bass_kernel_spmd(nc, in_maps, core_ids=list(range(8)))
    out = np.empty((B, DIM, HH, WW), dtype=np.float32)
    for b in range(B):
        part = (
            res.results[2 * b]["outp"].astype(np.float32)
            + res.results[2 * b + 1]["outp"].astype(np.float32)
        )
        out[b] = part.reshape(DIM, HH, WW)
    return out
